# revision 31
# baseline (speedup 1.0000x reference)
"""Multi-head attention kernel for Trainium2, data-parallel over batch on 8 cores.

Problem: B=16, N=1024, DIM=768, H=12 heads, head_dim=64, fp32.
  q = x@Wq+bq; k = x@Wk+bk; v = x@Wv+bv   (per-head split)
  out = softmax(q k^T / sqrt(DIM)) v      (per head), concat, @Wo + bo

Sharding: batch-parallel. Each core gets 2 batches and all weights; no
collectives. Output gathered by concat.

Per-core layout strategy (per batch of 1024 tokens):
  - XT = x^T  [768 feat, 1024 tok] via PE transposes (fp32 DMA transpose
    unsupported).
  - QT/KT = (x@W + b)^T [768, 1024]: matmul(lhsT=W, rhs=XT). Head h lives on
    partition rows (h%2)*64..: pair p = m-tile p.
  - V natural [1024 tok, 768] via matmul(lhsT=XT, rhs=Wv), stored per-pair
    padded: [Vh0(64) | ones(1) | pad(31) | Vh1(64)] = 160 cols. The shared
    ones column makes PV emit softmax denominators at 32-aligned psum rows:
      h0: lhsT cols [0:128]  -> psum rows 0-63 = O_h0^T, row 64 = denom_h0
      h1: lhsT cols [32:160] -> psum row 32 = denom_h1, rows 64-127 = O_h1^T
         (remaining rows garbage, never read)
  - S^T[key, q] = matmul(lhsT=KT head rows, rhs=QT head rows), contraction 64,
    two heads row-packed in the PE array (partitions 0-63 / 64-127).
  - P^T = exp(SCALE * S^T) on ACT (no max subtraction needed: |SCALE*S| < ~2),
    [128, 1024] ops (2 key-blocks per op) to amortize ACT overhead.
  - O^T normalized by broadcast reciprocal rows, written to OT [768, 1024].
  - Y = matmul(lhsT=OT, rhs=Wo) + bo -> natural [tok, 768], DMA out.

All matmuls (projections and attention) run in bf16 with fp32 psum
accumulation: the PE moving-operand port is 2 B/lane/cycle on TRN2, so bf16
streams 1 col/cycle while f32r needs 2 — bf16 projections measure ~65us
faster end-to-end than f32r (434us vs 500us same-session), at rel err
~3.8e-3 of the output absmax (budget 2e-2). x is cast to bf16 in DRAM and
transposed via the XBAR DMA-transpose, so the PE does no transpose work.
"""

import sys
import types
from collections import deque

sys.path.insert(0, "/opt/trn_rl_repo")

import numpy as np

# Register the axon NTFF profile hook if the image's antenv lacks it (needed
# only when run with trace=True; harmless otherwise).
import antenv  # noqa: F401

if "antenv.axon_hooks" not in sys.modules:
    _hooks_mod = types.ModuleType("antenv.axon_hooks")
    _hooks_mod._hook = None

    def _set_hook(h):
        _hooks_mod._hook = h

    def _get_hook():
        return _hooks_mod._hook

    _hooks_mod.set_axon_ntff_profile_hook = _set_hook
    _hooks_mod.get_axon_ntff_profile_hook = _get_hook
    sys.modules["antenv.axon_hooks"] = _hooks_mod
    try:
        from trn_agent_boot.trn_boot import _ntff_profile_via_ctypes

        _set_hook(_ntff_profile_via_ctypes("/opt/axon/libaxon_pjrt.so"))
    except Exception:
        pass

import concourse.bass_utils as bass_utils

bass_utils.upload_artifacts = lambda tmpdir: f"local:{tmpdir}"  # no bucket creds

import concourse.bacc as bacc
import concourse.mybir as mybir
import concourse.tile as tile
from concourse.bass_utils import run_bass_kernel_spmd
from concourse.masks import make_identity

P = 128
DIM = 768
N_HEADS = 12
HD = 64
N = 1024
B = 16
NCORES = 8
BL = B // NCORES  # batches per core = 2
SCALE = 1.0 / float(np.sqrt(DIM))

KT = DIM // P      # 6 k-tiles of the 768 contraction
TT = N // P        # 8 token tiles per batch
NPAIR = N_HEADS // 2  # 6 head pairs
QC = 512           # query chunk (psum bank, fp32)
PAIRW = 160        # pair block in V_ext: [Vh0(64)|ones(1)|pad(31)|Vh1(64)]

F32 = mybir.dt.float32

_cache = {}


def build(mm_dtype, attn_bf16=True, dbg=False, warm=False):
    nc = bacc.Bacc("TRN2", target_bir_lowering=False, debug=False)

    x = nc.dram_tensor("inputs", [BL, N, DIM], F32, kind="ExternalInput")
    wq = nc.dram_tensor("Wq", [DIM, DIM], F32, kind="ExternalInput")
    bq = nc.dram_tensor("bq", [DIM], F32, kind="ExternalInput")
    wk = nc.dram_tensor("Wk", [DIM, DIM], F32, kind="ExternalInput")
    bk = nc.dram_tensor("bk", [DIM], F32, kind="ExternalInput")
    wv = nc.dram_tensor("Wv", [DIM, DIM], F32, kind="ExternalInput")
    bv = nc.dram_tensor("bv", [DIM], F32, kind="ExternalInput")
    wo = nc.dram_tensor("Wo", [DIM, DIM], F32, kind="ExternalInput")
    bo = nc.dram_tensor("bo", [DIM], F32, kind="ExternalInput")
    out = nc.dram_tensor("out", [BL, N, DIM], F32, kind="ExternalOutput")
    if dbg:
        d_xt = nc.dram_tensor("d_xt", [P, KT, N], F32, kind="ExternalOutput")
        d_vext = nc.dram_tensor("d_vext", [P, TT, NPAIR * PAIRW], F32, kind="ExternalOutput")
        d_qt = nc.dram_tensor("d_qt", [P, N], F32, kind="ExternalOutput")
        d_kt = nc.dram_tensor("d_kt", [P, N], F32, kind="ExternalOutput")
        d_pt0 = nc.dram_tensor("d_pt0", [P, 2 * QC], F32, kind="ExternalOutput")
        d_pt1 = nc.dram_tensor("d_pt1", [P, 2 * QC], F32, kind="ExternalOutput")
        d_osba = nc.dram_tensor("d_osba", [P, QC], F32, kind="ExternalOutput")
        d_osbb = nc.dram_tensor("d_osbb", [P, QC], F32, kind="ExternalOutput")
        d_rb = nc.dram_tensor("d_rb", [P, QC], F32, kind="ExternalOutput")
        d_ot = nc.dram_tensor("d_ot", [P, KT, N], F32, kind="ExternalOutput")

    wq_r = wq.rearrange("(ko ki) m -> ki ko m", ki=P)
    wk_r = wk.rearrange("(ko ki) m -> ki ko m", ki=P)
    wv_r = wv.rearrange("(ko ki) m -> ki ko m", ki=P)
    wo_r = wo.rearrange("(ko ki) m -> ki ko m", ki=P)
    bq_r = bq.rearrange("(ko ki) -> ki ko", ki=P)
    bk_r = bk.rearrange("(ko ki) -> ki ko", ki=P)

    # weights DMA: gpsimd can cast f32 -> f32r/bf16 in flight
    wdma = nc.sync.dma_start if mm_dtype == F32 else nc.gpsimd.dma_start
    attn_dt = mybir.dt.bfloat16 if attn_bf16 else mm_dtype
    xbufs = 3 if mm_dtype == mybir.dt.bfloat16 else 2

    with tile.TileContext(nc) as tc:
        with (
            tc.tile_pool(name="const", bufs=1) as cpool,
            tc.tile_pool(name="work", bufs=1) as pool,
            tc.tile_pool(name="dram", bufs=1, space="DRAM") as dpool,
            tc.tile_pool(name="ps", bufs=1, space="PSUM") as ps,
        ):
            ident = cpool.tile([P, P], F32)
            make_identity(nc, ident)

            if warm:
                # dummy matmuls to hold the DVFS p-state up while the x cast
                # + XBAR transpose + weight-load prologue runs (PE otherwise
                # idles at kernel start and the clock halves). The pipelined
                # per-token-block transposes get real work going ~12us in, so
                # 32 warm matmuls suffice to bridge.
                wrm = cpool.tile([P, QC], mybir.dt.bfloat16)
                nc.vector.memset(wrm[:], 0.0)
                for _ in range(16):
                    wps = ps.tile([P, QC], F32, tag="mm", bufs=2, name="wps")
                    nc.tensor.matmul(
                        wps[:], wrm[:, 0:P], wrm[:], start=True, stop=True
                    )

            bf16_x = mm_dtype == mybir.dt.bfloat16

            # resident weights (full); order by first use. With x loaded on
            # the sync queue (below), the gpsimd cast queue carries ONLY the
            # weights, so wv lands ~9us in and wq/wk by ~23us.
            wq_sb = cpool.tile([P, KT, DIM], mm_dtype)
            wk_sb = cpool.tile([P, KT, DIM], mm_dtype)
            wv_sb = cpool.tile([P, KT, DIM], mm_dtype)
            wo_sb = cpool.tile([P, KT, DIM], mm_dtype)
            for k in range(KT):
                wdma(wv_sb[:, k], wv_r[:, k])
            for k in range(KT):
                wdma(wq_sb[:, k], wq_r[:, k])
                wdma(wk_sb[:, k], wk_r[:, k])
            for k in range(KT):
                wdma(wo_sb[:, k], wo_r[:, k])

            bq_sb = cpool.tile([P, KT], F32)
            bk_sb = cpool.tile([P, KT], F32)
            bv_b = cpool.tile([P, DIM], F32)
            bo_b = cpool.tile([P, DIM], F32)

            # V_ext: [tok_inner, tok_outer, pair blocks of PAIRW cols]
            # cols p*PAIRW + [0:64] = V head 2p, +64 = ones, +[96:160] = V 2p+1
            # pad cols stay uninitialized: they only produce garbage psum rows
            # that are never read. Ones col via DVE cast-copy (f32r producer).
            # Double-buffered (one per batch): b1's V projection runs as
            # filler DURING b0's attention, which still reads b0's V.
            ones_src = cpool.tile([P, TT * NPAIR], F32)
            nc.vector.memset(ones_src[:], 1.0)

            def _alloc_vext():
                v_ext = pool.tile(
                    [P, TT, NPAIR * PAIRW], attn_dt, tag="v_ext", bufs=2,
                    name="v_ext",
                )
                ones_cols = v_ext[:].rearrange(
                    "p t (np w) -> p t np w", w=PAIRW
                )[:, :, :, 64:65]
                nc.vector.tensor_copy(
                    ones_cols,
                    ones_src[:].rearrange("p (t np) -> p t np", np=NPAIR)[
                        :, :, :, None
                    ],
                )
                return v_ext

            # XT via PE transposes: x token blocks stream in as plain f32 on
            # the otherwise-idle sync queue (no cast round-trip through DRAM,
            # no slow small-block XBAR transposes); the PE transposes each
            # 128x128 chunk (f32 transpose = 2 cycles/row, ~107ns/chunk, and
            # doubles as DVFS warm-up work) and the DVE copies psum -> xt in
            # bf16. Both batches run here: b1's transposes fill the wv-paced
            # bubbles of b0's V projection, and issuing its x loads late
            # would park them behind b0's attention-epilogue DMAs (FIFO).
            xts = []

            def _transpose_x(b):
                # psum -> xt copies: batch 0's ride the Scalar (ACT) engine,
                # which is idle until the first exp ~35us in; batch 1's MUST
                # NOT — the Scalar queue is FIFO, and b1's copies pace behind
                # b1's x loads, head-blocking every exp behind them (observed
                # as the whole attention pipeline stalling until ~45us). b1's
                # go to the DVE, which has no critical work before ~45us.
                # (gpsimd cannot read PSUM — NEFF compile rejects it.)
                # tp rides the "st" psum tag (idle until attention): on the
                # "mm" tag its WAR rotation against the V-chain accumulators
                # serializes transpose(to+1) behind V-chain(to)'s scatter,
                # stretching the whole prologue to ~60us.
                for to in range(TT):
                    tsl = slice(to * P, (to + 1) * P)
                    xstage = pool.tile([P, DIM], F32, tag="xstage", bufs=3)
                    nc.sync.dma_start(xstage[:], x[b, tsl, :])
                    tp = ps.tile([P, 2 * QC], F32, tag="st", bufs=2, name="tp")
                    for fo in range(KT):
                        nc.tensor.transpose(
                            tp[:, fo * P : (fo + 1) * P],
                            xstage[:, fo * P : (fo + 1) * P],
                            ident,
                        )
                    if b == 0:
                        nc.scalar.activation(
                            xts[b][:, 0:4, tsl],
                            tp[:, 0:512].rearrange("p (f t) -> p f t", f=4),
                            mybir.ActivationFunctionType.Copy,
                        )
                        nc.scalar.activation(
                            xts[b][:, 4:6, tsl],
                            tp[:, 512:768].rearrange("p (f t) -> p f t", f=2),
                            mybir.ActivationFunctionType.Copy,
                        )
                    else:
                        nc.vector.tensor_copy(
                            xts[b][:, 0:4, tsl],
                            tp[:, 0:512].rearrange("p (f t) -> p f t", f=4),
                        )
                        nc.vector.tensor_copy(
                            xts[b][:, 4:6, tsl],
                            tp[:, 512:768].rearrange("p (f t) -> p f t", f=2),
                        )

            for b in range(BL):
                xt_b = pool.tile(
                    [P, KT, N], mm_dtype, tag="xt_ot", bufs=xbufs, name=f"xt{b}"
                )
                xts.append(xt_b)
            _transpose_x(0)
            # b1's transposes are emitted later (inside b0's V-proj
            # phase) so they fill that phase's DMA-paced PE bubbles
            # instead of competing with b0's transposes up front.

            # ---- deferred-work machinery ------------------------------------
            # All projection work is expressed as closures ("items") grouped
            # into CHAINS (one psum accumulation each, ~6 items of ~2 matmuls).
            # Chains from the `bonus` queue fill attention exp-holes that the
            # pair-critical QK items don't cover; an open chain is finished
            # before anything else is popped so its psum-tag buffers (tag
            # "mm", 2 bufs) never interleave with another accumulation.
            bonus = deque()
            state = {"chain": None}

            def pop_filler(crit):
                ch = state["chain"]
                if ch is not None:
                    ch.popleft()()
                    if not ch:
                        state["chain"] = None
                    return
                if crit:
                    crit.popleft()()
                    return
                if bonus:
                    chain = bonus.popleft()
                    chain.popleft()()
                    state["chain"] = chain if chain else None

            def drain(crit):
                while state["chain"] is not None or crit:
                    pop_filler(crit)

            def drain_bonus():
                while state["chain"] is not None or bonus:
                    pop_filler([])

            def make_v_work(xt_b, vext_b):
                """V projection for one batch: one chain per token block
                (6 k-step items of 2 matmuls; the last scatters heads into
                the pair-padded v_ext layout and adds the bias)."""
                chains = []
                for to in range(TT):
                    holder = {}

                    def step(k, to=to, holder=holder):
                        if k == 0:
                            holder["vps"] = {
                                ch: ps.tile(
                                    [P, QC], F32, tag="mm", bufs=2, name=f"vps{ch}"
                                )
                                for ch in (0, 1)
                            }
                        for ch, cw in ((0, 512), (1, 256)):
                            nc.tensor.matmul(
                                holder["vps"][ch][:, :cw],
                                xt_b[:, k, to * P : (to + 1) * P],
                                wv_sb[:, k, ch * 512 : ch * 512 + cw],
                                start=(k == 0),
                                stop=(k == KT - 1),
                            )
                        if k != KT - 1:
                            return
                        for ch, cw in ((0, 512), (1, 256)):
                            vps = holder["vps"][ch]
                            npr = cw // (2 * HD)  # pairs in this chunk
                            pr0 = ch * 4
                            for par in (0, 1):  # even/odd head of each pair
                                src = vps[:, :cw].rearrange(
                                    "p (np two w) -> p np two w", two=2, w=HD
                                )[:, :, par, :]
                                bsrc = bv_b[:, ch * 512 : ch * 512 + cw].rearrange(
                                    "p (np two w) -> p np two w", two=2, w=HD
                                )[:, :, par, :]
                                off = 96 if par else 0
                                dst = vext_b[:, to, :].rearrange(
                                    "p (np w) -> p np w", w=PAIRW
                                )[:, pr0 : pr0 + npr, off : off + HD]
                                nc.vector.scalar_tensor_tensor(
                                    out=dst,
                                    in0=src,
                                    scalar=1.0,
                                    in1=bsrc,
                                    op0=mybir.AluOpType.mult,
                                    op1=mybir.AluOpType.add,
                                )

                    chains.append(
                        deque(lambda k=k, step=step: step(k) for k in range(KT))
                    )
                return chains

            def make_qk_work(xt_b, npo, nqt, nkt, ptags=("mm", "mm")):
                """QK projection for pair npo: two chains (dst q, dst k) of
                6 k-step items; the last item adds the biases. ptags picks
                the psum tags for the two query-chunk accumulators — pair 0
                of b0 runs pre-attention and borrows the idle oa/ob banks so
                it overlaps the V projection instead of serializing behind
                it on the mm tag's WAR rotation."""
                chains = []
                for dst_t, w_t, bias in ((nqt, wq_sb, bq_sb), (nkt, wk_sb, bk_sb)):
                    holder = {}

                    def chunk(k, dst_t=dst_t, w_t=w_t, bias=bias, holder=holder):
                        if k == 0:
                            holder["pp"] = [
                                ps.tile(
                                    [P, QC], F32, tag=ptags[qs],
                                    bufs=2 if ptags[qs] == "mm" else 1,
                                    name=f"pps{qs}",
                                )
                                for qs in range(N // QC)
                            ]
                        for qs in range(N // QC):
                            nc.tensor.matmul(
                                holder["pp"][qs][:],
                                w_t[:, k, npo * P : (npo + 1) * P],
                                xt_b[:, k, qs * QC : (qs + 1) * QC],
                                start=(k == 0),
                                stop=(k == KT - 1),
                            )
                        if k == KT - 1:
                            for qs in range(N // QC):
                                nc.vector.tensor_scalar_add(
                                    dst_t[:, qs * QC : (qs + 1) * QC],
                                    holder["pp"][qs][:],
                                    bias[:, npo : npo + 1],
                                )

                    chains.append(
                        deque(lambda k=k, chunk=chunk: chunk(k) for k in range(KT))
                    )
                return chains

            def make_y_work(b, ot_b):
                """Output projection for one batch: one chain per token block.
                Output DMA rides the gpsimd queue (idle post-prologue) so it
                can't delay the attention-epilogue DMAs on the sync queue."""
                chains = []
                for to in range(TT):
                    holder = {}

                    def step(k, b=b, ot_b=ot_b, to=to, holder=holder):
                        if k == 0:
                            holder["yps"] = {
                                ch: ps.tile(
                                    [P, QC], F32, tag="mm", bufs=2, name=f"yps{ch}"
                                )
                                for ch in (0, 1)
                            }
                        for ch, cw in ((0, 512), (1, 256)):
                            nc.tensor.matmul(
                                holder["yps"][ch][:, :cw],
                                ot_b[:, k, to * P : (to + 1) * P],
                                wo_sb[:, k, ch * 512 : ch * 512 + cw],
                                start=(k == 0),
                                stop=(k == KT - 1),
                            )
                        if k != KT - 1:
                            return
                        ystage = pool.tile([P, DIM], F32, tag="ystage", bufs=xbufs)
                        for ch, cw in ((0, 512), (1, 256)):
                            nc.vector.scalar_tensor_tensor(
                                out=ystage[:, ch * 512 : ch * 512 + cw],
                                in0=holder["yps"][ch][:, :cw],
                                scalar=1.0,
                                in1=bo_b[:, ch * 512 : ch * 512 + cw],
                                op0=mybir.AluOpType.mult,
                                op1=mybir.AluOpType.add,
                            )
                        nc.gpsimd.dma_start(
                            out[b, to * P : (to + 1) * P, :], ystage[:]
                        )

                    chains.append(
                        deque(lambda k=k, step=step: step(k) for k in range(KT))
                    )
                return chains

            pending_y = None
            b1_qkt = None
            for b in range(BL):
                xt = xts[b]
                if b == 0:
                    nc.scalar.dma_start(bv_b[:], bv[None, :].to_broadcast((P, DIM)))
                    nc.scalar.dma_start(bq_sb[:], bq_r)
                    nc.scalar.dma_start(bk_sb[:], bk_r)
                    nc.scalar.dma_start(bo_b[:], bo[None, :].to_broadcast((P, DIM)))
                    # ---- V natural (b0, inline) + b1 transposes -----------
                    v_ext = _alloc_vext()
                    for chain in make_v_work(xt, v_ext):
                        while chain:
                            chain.popleft()()
                    _transpose_x(1)
                    # pair 0's QK projection inline, on the idle oa/ob psum
                    # banks so it overlaps the V projection
                    qt_t = pool.tile([P, N], attn_dt, tag="qt", bufs=4)
                    kt_t = pool.tile([P, N], attn_dt, tag="kt", bufs=4)
                    for chain in make_qk_work(xt, 0, qt_t, kt_t, ("oa", "ob")):
                        while chain:
                            chain.popleft()()
                    # bonus for b0's attention: b1's V projection + b1's
                    # pair-0 QK projection
                    v_ext1 = _alloc_vext()
                    bonus.extend(make_v_work(xts[1], v_ext1))
                    b1_qkt = (
                        pool.tile([P, N], attn_dt, tag="qt", bufs=4, name="qt1"),
                        pool.tile([P, N], attn_dt, tag="kt", bufs=4, name="kt1"),
                    )
                    bonus.extend(make_qk_work(xts[1], 0, *b1_qkt))
                    if dbg:
                        nc.sync.dma_start(d_xt[:], xt[:].bitcast(F32))
                        nc.sync.dma_start(d_vext[:], v_ext[:].bitcast(F32))
                else:
                    # leftover b1 V / QK0 bonus must land before b1's
                    # attention reads v_ext1 / its qt,kt
                    drain_bonus()
                    v_ext = v_ext1
                    qt_t, kt_t = b1_qkt
                    # bonus for b1's attention: b0's output projection
                    bonus.extend(pending_y)

                # ---- OT buffer for this batch -----------------------------
                ot = pool.tile([P, KT, N], mm_dtype, tag="xt_ot", bufs=xbufs, name="ot")

                for po in range(NPAIR):
                    qk_work = deque()
                    if po + 1 < NPAIR:
                        nqt = pool.tile([P, N], attn_dt, tag="qt", bufs=4)
                        nkt = pool.tile([P, N], attn_dt, tag="kt", bufs=4)
                        for chain in make_qk_work(xt, po + 1, nqt, nkt):
                            qk_work.extend(chain)
                        next_tiles = (nqt, nkt)

                    if dbg and b == 0 and po == 0:
                        nc.sync.dma_start(d_qt[:], qt_t[:].bitcast(F32))
                        nc.sync.dma_start(d_kt[:], kt_t[:].bitcast(F32))

                    pb = po * PAIRW
                    for qc in range(N // QC):
                        qsl = slice(qc * QC, (qc + 1) * QC)
                        oa = ps.tile([P, QC], F32, tag="oa", bufs=1, name="oa")
                        ob = ps.tile([P, QC], F32, tag="ob", bufs=1, name="ob")
                        for kb in range(TT):
                            ksl = slice(kb * P, (kb + 1) * P)
                            # ONE st tile per key block holding BOTH heads
                            # ([h0 512 | h1 512]) so a single exp frees both
                            # banks at once: the two score matmuls then become
                            # ready together and issue back-to-back, streaming
                            # CONCURRENTLY on the two 64-row PE tile halves
                            # (h0 rows 0-63, h1 rows 64-127) — 2 cols/cycle.
                            # With separate st0/st1 + two exps, h1's bank
                            # frees ~1.1us after h0's and the scheduler runs
                            # the h0 matmuls solo, breaking the pairing.
                            st = ps.tile([P, 2 * QC], F32, tag="st", bufs=2, name="st")
                            # high_priority keeps the pair adjacent in the
                            # final schedule: filler k-steps become ready
                            # one-at-a-time (chained psum accumulation) and
                            # would otherwise slip between h0 and h64 with
                            # their lower (earlier) priorities.
                            with tc.high_priority(offset=256):
                                nc.tensor.matmul(
                                    st[:, 0:QC],
                                    kt_t[0:64, ksl],
                                    qt_t[0:64, qsl],
                                    start=True,
                                    stop=True,
                                )
                                nc.tensor.matmul(
                                    st[:, QC : 2 * QC],
                                    kt_t[64:128, ksl],
                                    qt_t[64:128, qsl],
                                    start=True,
                                    stop=True,
                                )
                            pt = pool.tile([P, 2 * QC], attn_dt, tag="pt0", bufs=xbufs)
                            nc.scalar.activation(
                                pt[:], st[:], mybir.ActivationFunctionType.Exp,
                                scale=SCALE,
                            )
                            if dbg and b == 0 and po == 0 and qc == 0 and kb == 0:
                                nc.sync.dma_start(d_pt0[:], pt[:].bitcast(F32))
                            first = kb == 0
                            last = kb == TT - 1
                            nc.tensor.matmul(
                                oa[:, :],
                                v_ext[:, kb, pb : pb + 128],
                                pt[:, 0:QC],
                                start=first,
                                stop=last,
                            )
                            nc.tensor.matmul(
                                ob[:, :],
                                v_ext[:, kb, pb + 32 : pb + 160],
                                pt[:, QC : 2 * QC],
                                start=first,
                                stop=last,
                            )
                            # splice filler into the per-key-block exp hole:
                            # exp (1114ns) exceeds this slot's PE work
                            # (paired scores ~240ns + 2 PV matmuls ~450ns),
                            # leaving ~420ns for one item (~2 matmuls) of
                            # next-pair QK proj (critical) or bonus work.
                            pop_filler(qk_work)
                        # epilogue: copy psum out early (frees oa/ob banks),
                        # then normalize by the ones-row sums
                        osb_a = pool.tile([P, QC], F32, tag="osb_a", bufs=xbufs)
                        osb_b = pool.tile([P, QC], F32, tag="osb_b", bufs=xbufs)
                        nc.vector.tensor_copy(osb_a[0:65, :], oa[0:65, :])
                        nc.vector.tensor_copy(osb_b[64:128, :], ob[64:128, :])
                        nc.vector.tensor_copy(osb_b[32:33, :], ob[32:33, :])
                        # denominators -> DRAM, reshaped to [128, 8] so the
                        # slow iterative DVE reciprocal uses all lanes, then
                        # broadcast back from DRAM (DMA partition-broadcast).
                        dden = dpool.tile([2, QC], F32, tag="dden", bufs=2)
                        nc.sync.dma_start(dden[0:1, :], osb_a[64:65, :])
                        nc.sync.dma_start(dden[1:2, :], osb_b[32:33, :])
                        den_sq = pool.tile([P, 8], F32, tag="den_sq", bufs=2)
                        nc.sync.dma_start(
                            den_sq[:],
                            dden[:].rearrange("a c -> (a c)").rearrange(
                                "(p f) -> p f", p=P
                            ),
                        )
                        rinv_sq = pool.tile([P, 8], F32, tag="rinv_sq", bufs=2)
                        nc.vector.reciprocal(rinv_sq[:], den_sq[:])
                        drin = dpool.tile([2, QC], F32, tag="drin", bufs=2)
                        nc.sync.dma_start(
                            drin[:].rearrange("a c -> (a c)").rearrange(
                                "(p f) -> p f", p=P
                            ),
                            rinv_sq[:],
                        )
                        rb = pool.tile([P, QC], F32, tag="rb", bufs=xbufs)
                        nc.sync.dma_start(
                            rb[0:64, :], drin[0:1, :].to_broadcast((64, QC))
                        )
                        nc.sync.dma_start(
                            rb[64:128, :], drin[1:2, :].to_broadcast((64, QC))
                        )
                        if dbg and b == 0 and po == 0 and qc == 0:
                            nc.sync.dma_start(d_osba[:], osb_a[:])
                            nc.sync.dma_start(d_osbb[:], osb_b[:])
                            nc.sync.dma_start(d_rb[:], rb[:])
                        nc.vector.tensor_mul(
                            ot[0:64, po, qsl], osb_a[0:64, :], rb[0:64, :]
                        )
                        nc.vector.tensor_mul(
                            ot[64:128, po, qsl], osb_b[64:128, :], rb[64:128, :]
                        )

                    drain(qk_work)
                    if po + 1 < NPAIR:
                        qt_t, kt_t = next_tiles

                if dbg and b == 0:
                    nc.sync.dma_start(d_ot[:], ot[:].bitcast(F32))

                # ---- Y = OT^T @ Wo + bo: deferred into the NEXT batch's
                # attention as bonus filler; the last batch's runs inline
                pending_y = make_y_work(b, ot)

            drain_bonus()
            for chain in pending_y:
                while chain:
                    chain.popleft()()

    nc.finalize()
    return nc


def _run(inputs: dict, mm_dtype=None, attn_bf16=True, trace: bool = False, dbg: bool = False):
    # bf16 projections beat f32r: the PE moving port is 2 B/lane/cycle, so
    # f32r streams at 2 cycles/col while bf16 streams at 1 (measured
    # ~434us bf16 vs ~500us f32r same-session; rel err 3.8e-3 vs 1.7e-3,
    # both far under the 2e-2 budget).
    if mm_dtype is None:
        mm_dtype = mybir.dt.bfloat16
    key = (str(mm_dtype), attn_bf16, dbg)
    if key not in _cache:
        # warm=True streams dummy matmuls under the load prologue so a
        # cold (down-clocked) device ramps the PE p-state before real work
        _cache[key] = build(mm_dtype, attn_bf16=attn_bf16, dbg=dbg, warm=True)
    nc = _cache[key]
    return _run_nc(nc, inputs, trace)


def _run_nc(nc, inputs: dict, trace: bool = False):

    x = np.ascontiguousarray(inputs["inputs"], dtype=np.float32)
    shared = {
        k: np.ascontiguousarray(inputs[k], dtype=np.float32)
        for k in ("Wq", "bq", "Wk", "bk", "Wv", "bv", "Wo", "bo")
    }
    in_maps = [
        {"inputs": x[c * BL : (c + 1) * BL], **shared} for c in range(NCORES)
    ]
    res = run_bass_kernel_spmd(nc, in_maps, list(range(NCORES)), trace=trace)
    full = np.concatenate([res.results[c]["out"] for c in range(NCORES)], axis=0)
    return full, res


def kernel(**inputs) -> np.ndarray:
    out, _ = _run(inputs)
    return out



# revision 35
# speedup vs baseline: 1.0413x; 1.0413x over previous
"""Multi-head attention kernel for Trainium2, data-parallel over batch on 8 cores.

Problem: B=16, N=1024, DIM=768, H=12 heads, head_dim=64, fp32.
  q = x@Wq+bq; k = x@Wk+bk; v = x@Wv+bv   (per-head split)
  out = softmax(q k^T / sqrt(DIM)) v      (per head), concat, @Wo + bo

Sharding: batch-parallel. Each core gets 2 batches and all weights; no
collectives. Output gathered by concat.

Per-core layout strategy (per batch of 1024 tokens):
  - XT = x^T  [768 feat, 1024 tok] via PE transposes (fp32 DMA transpose
    unsupported).
  - QT/KT = (x@W + b)^T [768, 1024]: matmul(lhsT=W, rhs=XT). Head h lives on
    partition rows (h%2)*64..: pair p = m-tile p.
  - V natural [1024 tok, 768] via matmul(lhsT=XT, rhs=Wv), stored per-pair
    padded: [Vh0(64) | ones(1) | pad(31) | Vh1(64)] = 160 cols. The shared
    ones column makes PV emit softmax denominators at 32-aligned psum rows:
      h0: lhsT cols [0:128]  -> psum rows 0-63 = O_h0^T, row 64 = denom_h0
      h1: lhsT cols [32:160] -> psum row 32 = denom_h1, rows 64-127 = O_h1^T
         (remaining rows garbage, never read)
  - S^T[key, q] = matmul(lhsT=KT head rows, rhs=QT head rows), contraction 64,
    two heads row-packed in the PE array (partitions 0-63 / 64-127).
  - P^T = exp(SCALE * S^T) on ACT (no max subtraction needed: |SCALE*S| < ~2),
    [128, 1024] ops (2 key-blocks per op) to amortize ACT overhead.
  - O^T normalized by broadcast reciprocal rows, written to OT [768, 1024].
  - Y = matmul(lhsT=OT, rhs=Wo) + bo -> natural [tok, 768], DMA out.

All matmuls (projections and attention) run in bf16 with fp32 psum
accumulation: the PE moving-operand port is 2 B/lane/cycle on TRN2, so bf16
streams 1 col/cycle while f32r needs 2 — bf16 projections measure ~65us
faster end-to-end than f32r (434us vs 500us same-session), at rel err
~3.8e-3 of the output absmax (budget 2e-2). x is cast to bf16 in DRAM and
transposed via the XBAR DMA-transpose, so the PE does no transpose work.
"""

import sys
import types
from collections import deque

sys.path.insert(0, "/opt/trn_rl_repo")

import numpy as np

# Register the axon NTFF profile hook if the image's antenv lacks it (needed
# only when run with trace=True; harmless otherwise).
import antenv  # noqa: F401

if "antenv.axon_hooks" not in sys.modules:
    _hooks_mod = types.ModuleType("antenv.axon_hooks")
    _hooks_mod._hook = None

    def _set_hook(h):
        _hooks_mod._hook = h

    def _get_hook():
        return _hooks_mod._hook

    _hooks_mod.set_axon_ntff_profile_hook = _set_hook
    _hooks_mod.get_axon_ntff_profile_hook = _get_hook
    sys.modules["antenv.axon_hooks"] = _hooks_mod
    try:
        from trn_agent_boot.trn_boot import _ntff_profile_via_ctypes

        _set_hook(_ntff_profile_via_ctypes("/opt/axon/libaxon_pjrt.so"))
    except Exception:
        pass

import concourse.bass_utils as bass_utils

bass_utils.upload_artifacts = lambda tmpdir: f"local:{tmpdir}"  # no bucket creds

import concourse.bacc as bacc
import concourse.mybir as mybir
import concourse.tile as tile
from concourse.bass_utils import run_bass_kernel_spmd
from concourse.masks import make_identity

P = 128
DIM = 768
N_HEADS = 12
HD = 64
N = 1024
B = 16
NCORES = 8
BL = B // NCORES  # batches per core = 2
SCALE = 1.0 / float(np.sqrt(DIM))

KT = DIM // P      # 6 k-tiles of the 768 contraction
TT = N // P        # 8 token tiles per batch
NPAIR = N_HEADS // 2  # 6 head pairs
QC = 512           # query chunk (psum bank, fp32)
PAIRW = 160        # pair block in V_ext: [Vh0(64)|ones(1)|pad(31)|Vh1(64)]

F32 = mybir.dt.float32

_cache = {}


def build(mm_dtype, attn_bf16=True, dbg=False, warm=False):
    nc = bacc.Bacc("TRN2", target_bir_lowering=False, debug=False)

    x = nc.dram_tensor("inputs", [BL, N, DIM], F32, kind="ExternalInput")
    wq = nc.dram_tensor("Wq", [DIM, DIM], F32, kind="ExternalInput")
    bq = nc.dram_tensor("bq", [DIM], F32, kind="ExternalInput")
    wk = nc.dram_tensor("Wk", [DIM, DIM], F32, kind="ExternalInput")
    bk = nc.dram_tensor("bk", [DIM], F32, kind="ExternalInput")
    wv = nc.dram_tensor("Wv", [DIM, DIM], F32, kind="ExternalInput")
    bv = nc.dram_tensor("bv", [DIM], F32, kind="ExternalInput")
    wo = nc.dram_tensor("Wo", [DIM, DIM], F32, kind="ExternalInput")
    bo = nc.dram_tensor("bo", [DIM], F32, kind="ExternalInput")
    out = nc.dram_tensor("out", [BL, N, DIM], F32, kind="ExternalOutput")
    if dbg:
        d_xt = nc.dram_tensor("d_xt", [P, KT, N], F32, kind="ExternalOutput")
        d_vext = nc.dram_tensor("d_vext", [P, TT, NPAIR * PAIRW], F32, kind="ExternalOutput")
        d_qt = nc.dram_tensor("d_qt", [P, N], F32, kind="ExternalOutput")
        d_kt = nc.dram_tensor("d_kt", [P, N], F32, kind="ExternalOutput")
        d_pt0 = nc.dram_tensor("d_pt0", [P, 2 * QC], F32, kind="ExternalOutput")
        d_pt1 = nc.dram_tensor("d_pt1", [P, 2 * QC], F32, kind="ExternalOutput")
        d_osba = nc.dram_tensor("d_osba", [P, QC], F32, kind="ExternalOutput")
        d_osbb = nc.dram_tensor("d_osbb", [P, QC], F32, kind="ExternalOutput")
        d_rb = nc.dram_tensor("d_rb", [P, QC], F32, kind="ExternalOutput")
        d_ot = nc.dram_tensor("d_ot", [P, KT, N], F32, kind="ExternalOutput")

    wq_r = wq.rearrange("(ko ki) m -> ki ko m", ki=P)
    wk_r = wk.rearrange("(ko ki) m -> ki ko m", ki=P)
    wv_r = wv.rearrange("(ko ki) m -> ki ko m", ki=P)
    wo_r = wo.rearrange("(ko ki) m -> ki ko m", ki=P)
    bq_r = bq.rearrange("(ko ki) -> ki ko", ki=P)
    bk_r = bk.rearrange("(ko ki) -> ki ko", ki=P)

    # weights DMA: gpsimd can cast f32 -> f32r/bf16 in flight
    wdma = nc.sync.dma_start if mm_dtype == F32 else nc.gpsimd.dma_start
    attn_dt = mybir.dt.bfloat16 if attn_bf16 else mm_dtype
    xbufs = 3 if mm_dtype == mybir.dt.bfloat16 else 2

    with tile.TileContext(nc) as tc:
        with (
            tc.tile_pool(name="const", bufs=1) as cpool,
            tc.tile_pool(name="work", bufs=1) as pool,
            tc.tile_pool(name="dram", bufs=1, space="DRAM") as dpool,
            tc.tile_pool(name="ps", bufs=1, space="PSUM") as ps,
        ):
            ident = cpool.tile([P, P], F32)
            make_identity(nc, ident)

            if warm:
                # dummy matmuls to hold the DVFS p-state up while the x cast
                # + XBAR transpose + weight-load prologue runs (PE otherwise
                # idles at kernel start and the clock halves). The pipelined
                # per-token-block transposes get real work going ~12us in, so
                # 32 warm matmuls suffice to bridge.
                wrm = cpool.tile([P, QC], mybir.dt.bfloat16)
                nc.vector.memset(wrm[:], 0.0)
                for _ in range(16):
                    wps = ps.tile([P, QC], F32, tag="mm", bufs=2, name="wps")
                    nc.tensor.matmul(
                        wps[:], wrm[:, 0:P], wrm[:], start=True, stop=True
                    )

            bf16_x = mm_dtype == mybir.dt.bfloat16

            # resident weights (full); order by first use. With x loaded on
            # the sync queue (below), the gpsimd cast queue carries ONLY the
            # weights, so wv lands ~9us in and wq/wk by ~23us.
            wq_sb = cpool.tile([P, KT, DIM], mm_dtype)
            wk_sb = cpool.tile([P, KT, DIM], mm_dtype)
            wv_sb = cpool.tile([P, KT, DIM], mm_dtype)
            wo_sb = cpool.tile([P, KT, DIM], mm_dtype)
            for k in range(KT):
                wdma(wv_sb[:, k], wv_r[:, k])
            for k in range(KT):
                wdma(wq_sb[:, k], wq_r[:, k])
                wdma(wk_sb[:, k], wk_r[:, k])
            # b1's x: cast to bf16 in DRAM (gpsimd, after the weights b0's
            # prologue needs) then ONE full-batch XBAR transpose — zero PE
            # cost, lands ~38us, in time for the V-b1 bonus chains. (Per-
            # token-block XBAR transposes run ~10x slower per byte — the
            # write stripes shrink to 256B — so only the full-batch form is
            # usable, and b0 can't wait for it.)
            xbf1 = None
            if mm_dtype == mybir.dt.bfloat16:
                xbf1 = dpool.tile([N, DIM], mybir.dt.bfloat16, name="xbf1")
                for to in range(TT):
                    nc.gpsimd.dma_start(
                        xbf1[to * P : (to + 1) * P, :],
                        x[1, to * P : (to + 1) * P, :],
                    )
            for k in range(KT):
                wdma(wo_sb[:, k], wo_r[:, k])

            bq_sb = cpool.tile([P, KT], F32)
            bk_sb = cpool.tile([P, KT], F32)
            bv_b = cpool.tile([P, DIM], F32)
            bo_b = cpool.tile([P, DIM], F32)

            # V_ext: [tok_inner, tok_outer, pair blocks of PAIRW cols]
            # cols p*PAIRW + [0:64] = V head 2p, +64 = ones, +[96:160] = V 2p+1
            # pad cols stay uninitialized: they only produce garbage psum rows
            # that are never read. Ones col via DVE cast-copy (f32r producer).
            # Double-buffered (one per batch): b1's V projection runs as
            # filler DURING b0's attention, which still reads b0's V.
            ones_src = cpool.tile([P, TT * NPAIR], F32)
            nc.vector.memset(ones_src[:], 1.0)

            def _alloc_vext():
                v_ext = pool.tile(
                    [P, TT, NPAIR * PAIRW], attn_dt, tag="v_ext", bufs=2,
                    name="v_ext",
                )
                ones_cols = v_ext[:].rearrange(
                    "p t (np w) -> p t np w", w=PAIRW
                )[:, :, :, 64:65]
                nc.vector.tensor_copy(
                    ones_cols,
                    ones_src[:].rearrange("p (t np) -> p t np", np=NPAIR)[
                        :, :, :, None
                    ],
                )
                return v_ext

            # XT via PE transposes: x token blocks stream in as plain f32 on
            # the otherwise-idle sync queue (no cast round-trip through DRAM,
            # no slow small-block XBAR transposes); the PE transposes each
            # 128x128 chunk (f32 transpose = 2 cycles/row, ~107ns/chunk, and
            # doubles as DVFS warm-up work) and the DVE copies psum -> xt in
            # bf16. Both batches run here: b1's transposes fill the wv-paced
            # bubbles of b0's V projection, and issuing its x loads late
            # would park them behind b0's attention-epilogue DMAs (FIFO).
            xts = []

            def _transpose_x(b):
                # psum -> xt copies ride the Scalar (ACT) engine, idle until
                # the first exp ~35us in. tp rides the "st" psum tag (also
                # idle until attention): on the "mm" tag its WAR rotation
                # against the V-chain accumulators serializes transpose(to+1)
                # behind V-chain(to)'s scatter, stretching the prologue.
                for to in range(TT):
                    tsl = slice(to * P, (to + 1) * P)
                    xstage = pool.tile([P, DIM], F32, tag="xstage", bufs=3)
                    nc.sync.dma_start(xstage[:], x[b, tsl, :])
                    tp = ps.tile([P, 2 * QC], F32, tag="st", bufs=2, name="tp")
                    for fo in range(KT):
                        nc.tensor.transpose(
                            tp[:, fo * P : (fo + 1) * P],
                            xstage[:, fo * P : (fo + 1) * P],
                            ident,
                        )
                    nc.scalar.activation(
                        xts[b][:, 0:4, tsl],
                        tp[:, 0:512].rearrange("p (f t) -> p f t", f=4),
                        mybir.ActivationFunctionType.Copy,
                    )
                    nc.scalar.activation(
                        xts[b][:, 4:6, tsl],
                        tp[:, 512:768].rearrange("p (f t) -> p f t", f=2),
                        mybir.ActivationFunctionType.Copy,
                    )

            for b in range(BL):
                xt_b = pool.tile(
                    [P, KT, N], mm_dtype, tag="xt_ot", bufs=xbufs, name=f"xt{b}"
                )
                xts.append(xt_b)
            _transpose_x(0)
            if xbf1 is not None:
                # b1's transpose: XBAR, no PE cost. Queued on sync after b0's
                # x loads; it waits on the b1 cast (~30us) while the first
                # attention-epilogue DMA isn't needed until ~50us.
                nc.sync.dma_start_transpose(xts[1][:], xbf1[:])
            else:
                _transpose_x(1)

            # ---- deferred-work machinery ------------------------------------
            # All projection work is expressed as closures ("items") grouped
            # into CHAINS (one psum accumulation each, ~6 items of ~2 matmuls).
            # Chains from the `bonus` queue fill attention exp-holes that the
            # pair-critical QK items don't cover; an open chain is finished
            # before anything else is popped so its psum-tag buffers (tag
            # "mm", 2 bufs) never interleave with another accumulation.
            bonus = deque()
            state = {"chain": None}

            def pop_filler(crit):
                ch = state["chain"]
                if ch is not None:
                    ch.popleft()()
                    if not ch:
                        state["chain"] = None
                    return
                if crit:
                    crit.popleft()()
                    return
                if bonus:
                    chain = bonus.popleft()
                    chain.popleft()()
                    state["chain"] = chain if chain else None

            def drain(crit):
                while state["chain"] is not None or crit:
                    pop_filler(crit)

            def drain_bonus():
                while state["chain"] is not None or bonus:
                    pop_filler([])

            def make_v_work(xt_b, vext_b):
                """V projection for one batch: one chain per token block
                (6 k-step items of 2 matmuls; the last scatters heads into
                the pair-padded v_ext layout and adds the bias)."""
                chains = []
                for to in range(TT):
                    holder = {}

                    def step(k, to=to, holder=holder):
                        if k == 0:
                            holder["vps"] = {
                                ch: ps.tile(
                                    [P, QC], F32, tag="mm", bufs=2, name=f"vps{ch}"
                                )
                                for ch in (0, 1)
                            }
                        for ch, cw in ((0, 512), (1, 256)):
                            nc.tensor.matmul(
                                holder["vps"][ch][:, :cw],
                                xt_b[:, k, to * P : (to + 1) * P],
                                wv_sb[:, k, ch * 512 : ch * 512 + cw],
                                start=(k == 0),
                                stop=(k == KT - 1),
                            )
                        if k != KT - 1:
                            return
                        for ch, cw in ((0, 512), (1, 256)):
                            vps = holder["vps"][ch]
                            npr = cw // (2 * HD)  # pairs in this chunk
                            pr0 = ch * 4
                            for par in (0, 1):  # even/odd head of each pair
                                src = vps[:, :cw].rearrange(
                                    "p (np two w) -> p np two w", two=2, w=HD
                                )[:, :, par, :]
                                bsrc = bv_b[:, ch * 512 : ch * 512 + cw].rearrange(
                                    "p (np two w) -> p np two w", two=2, w=HD
                                )[:, :, par, :]
                                off = 96 if par else 0
                                dst = vext_b[:, to, :].rearrange(
                                    "p (np w) -> p np w", w=PAIRW
                                )[:, pr0 : pr0 + npr, off : off + HD]
                                nc.vector.scalar_tensor_tensor(
                                    out=dst,
                                    in0=src,
                                    scalar=1.0,
                                    in1=bsrc,
                                    op0=mybir.AluOpType.mult,
                                    op1=mybir.AluOpType.add,
                                )

                    chains.append(
                        deque(lambda k=k, step=step: step(k) for k in range(KT))
                    )
                return chains

            def make_qk_work(xt_b, npo, nqt, nkt, ptags=("mm", "mm")):
                """QK projection for pair npo: two chains (dst q, dst k) of
                6 k-step items; the last item adds the biases. ptags picks
                the psum tags for the two query-chunk accumulators — pair 0
                of b0 runs pre-attention and borrows the idle oa/ob banks so
                it overlaps the V projection instead of serializing behind
                it on the mm tag's WAR rotation."""
                chains = []
                for dst_t, w_t, bias in ((nqt, wq_sb, bq_sb), (nkt, wk_sb, bk_sb)):
                    holder = {}

                    def chunk(k, dst_t=dst_t, w_t=w_t, bias=bias, holder=holder):
                        if k == 0:
                            holder["pp"] = [
                                ps.tile(
                                    [P, QC], F32, tag=ptags[qs],
                                    bufs=2 if ptags[qs] == "mm" else 1,
                                    name=f"pps{qs}",
                                )
                                for qs in range(N // QC)
                            ]
                        for qs in range(N // QC):
                            nc.tensor.matmul(
                                holder["pp"][qs][:],
                                w_t[:, k, npo * P : (npo + 1) * P],
                                xt_b[:, k, qs * QC : (qs + 1) * QC],
                                start=(k == 0),
                                stop=(k == KT - 1),
                            )
                        if k == KT - 1:
                            for qs in range(N // QC):
                                nc.vector.tensor_scalar_add(
                                    dst_t[:, qs * QC : (qs + 1) * QC],
                                    holder["pp"][qs][:],
                                    bias[:, npo : npo + 1],
                                )

                    chains.append(
                        deque(lambda k=k, chunk=chunk: chunk(k) for k in range(KT))
                    )
                return chains

            def make_y_work(b, ot_b):
                """Output projection for one batch: one chain per token block.
                Output DMA rides the gpsimd queue (idle post-prologue) so it
                can't delay the attention-epilogue DMAs on the sync queue."""
                chains = []
                for to in range(TT):
                    holder = {}

                    def step(k, b=b, ot_b=ot_b, to=to, holder=holder):
                        if k == 0:
                            holder["yps"] = {
                                ch: ps.tile(
                                    [P, QC], F32, tag="mm", bufs=2, name=f"yps{ch}"
                                )
                                for ch in (0, 1)
                            }
                        for ch, cw in ((0, 512), (1, 256)):
                            nc.tensor.matmul(
                                holder["yps"][ch][:, :cw],
                                ot_b[:, k, to * P : (to + 1) * P],
                                wo_sb[:, k, ch * 512 : ch * 512 + cw],
                                start=(k == 0),
                                stop=(k == KT - 1),
                            )
                        if k != KT - 1:
                            return
                        ystage = pool.tile([P, DIM], F32, tag="ystage", bufs=xbufs)
                        for ch, cw in ((0, 512), (1, 256)):
                            nc.vector.scalar_tensor_tensor(
                                out=ystage[:, ch * 512 : ch * 512 + cw],
                                in0=holder["yps"][ch][:, :cw],
                                scalar=1.0,
                                in1=bo_b[:, ch * 512 : ch * 512 + cw],
                                op0=mybir.AluOpType.mult,
                                op1=mybir.AluOpType.add,
                            )
                        nc.gpsimd.dma_start(
                            out[b, to * P : (to + 1) * P, :], ystage[:]
                        )

                    chains.append(
                        deque(lambda k=k, step=step: step(k) for k in range(KT))
                    )
                return chains

            pending_y = None
            b1_qkt = None
            for b in range(BL):
                xt = xts[b]
                if b == 0:
                    nc.scalar.dma_start(bv_b[:], bv[None, :].to_broadcast((P, DIM)))
                    nc.scalar.dma_start(bq_sb[:], bq_r)
                    nc.scalar.dma_start(bk_sb[:], bk_r)
                    nc.scalar.dma_start(bo_b[:], bo[None, :].to_broadcast((P, DIM)))
                    # ---- V natural (b0, inline) ---------------------------
                    v_ext = _alloc_vext()
                    for chain in make_v_work(xt, v_ext):
                        while chain:
                            chain.popleft()()
                    # pair 0's QK projection inline, on the idle oa/ob psum
                    # banks so it overlaps the V projection
                    qt_t = pool.tile([P, N], attn_dt, tag="qt", bufs=4)
                    kt_t = pool.tile([P, N], attn_dt, tag="kt", bufs=4)
                    for chain in make_qk_work(xt, 0, qt_t, kt_t, ("oa", "ob")):
                        while chain:
                            chain.popleft()()
                    # bonus for b0's attention: b1's V projection + b1's
                    # pair-0 QK projection
                    v_ext1 = _alloc_vext()
                    bonus.extend(make_v_work(xts[1], v_ext1))
                    b1_qkt = (
                        pool.tile([P, N], attn_dt, tag="qt", bufs=4, name="qt1"),
                        pool.tile([P, N], attn_dt, tag="kt", bufs=4, name="kt1"),
                    )
                    bonus.extend(make_qk_work(xts[1], 0, *b1_qkt))
                    if dbg:
                        nc.sync.dma_start(d_xt[:], xt[:].bitcast(F32))
                        nc.sync.dma_start(d_vext[:], v_ext[:].bitcast(F32))
                else:
                    # leftover b1 V / QK0 bonus must land before b1's
                    # attention reads v_ext1 / its qt,kt
                    drain_bonus()
                    v_ext = v_ext1
                    qt_t, kt_t = b1_qkt
                    # bonus for b1's attention: b0's output projection
                    bonus.extend(pending_y)

                # ---- OT buffer for this batch -----------------------------
                ot = pool.tile([P, KT, N], mm_dtype, tag="xt_ot", bufs=xbufs, name="ot")

                for po in range(NPAIR):
                    qk_work = deque()
                    if po + 1 < NPAIR:
                        nqt = pool.tile([P, N], attn_dt, tag="qt", bufs=4)
                        nkt = pool.tile([P, N], attn_dt, tag="kt", bufs=4)
                        for chain in make_qk_work(xt, po + 1, nqt, nkt):
                            qk_work.extend(chain)
                        next_tiles = (nqt, nkt)

                    if dbg and b == 0 and po == 0:
                        nc.sync.dma_start(d_qt[:], qt_t[:].bitcast(F32))
                        nc.sync.dma_start(d_kt[:], kt_t[:].bitcast(F32))

                    pb = po * PAIRW
                    for qc in range(N // QC):
                        qsl = slice(qc * QC, (qc + 1) * QC)
                        oa = ps.tile([P, QC], F32, tag="oa", bufs=1, name="oa")
                        ob = ps.tile([P, QC], F32, tag="ob", bufs=1, name="ob")
                        for kb in range(TT):
                            ksl = slice(kb * P, (kb + 1) * P)
                            # ONE st tile per key block holding BOTH heads
                            # ([h0 512 | h1 512]) so a single exp frees both
                            # banks at once: the two score matmuls then become
                            # ready together and issue back-to-back, streaming
                            # CONCURRENTLY on the two 64-row PE tile halves
                            # (h0 rows 0-63, h1 rows 64-127) — 2 cols/cycle.
                            # With separate st0/st1 + two exps, h1's bank
                            # frees ~1.1us after h0's and the scheduler runs
                            # the h0 matmuls solo, breaking the pairing.
                            st = ps.tile([P, 2 * QC], F32, tag="st", bufs=2, name="st")
                            # high_priority keeps the pair adjacent in the
                            # final schedule: filler k-steps become ready
                            # one-at-a-time (chained psum accumulation) and
                            # would otherwise slip between h0 and h64 with
                            # their lower (earlier) priorities.
                            with tc.high_priority(offset=256):
                                nc.tensor.matmul(
                                    st[:, 0:QC],
                                    kt_t[0:64, ksl],
                                    qt_t[0:64, qsl],
                                    start=True,
                                    stop=True,
                                )
                                nc.tensor.matmul(
                                    st[:, QC : 2 * QC],
                                    kt_t[64:128, ksl],
                                    qt_t[64:128, qsl],
                                    start=True,
                                    stop=True,
                                )
                            pt = pool.tile([P, 2 * QC], attn_dt, tag="pt0", bufs=xbufs)
                            nc.scalar.activation(
                                pt[:], st[:], mybir.ActivationFunctionType.Exp,
                                scale=SCALE,
                            )
                            if dbg and b == 0 and po == 0 and qc == 0 and kb == 0:
                                nc.sync.dma_start(d_pt0[:], pt[:].bitcast(F32))
                            first = kb == 0
                            last = kb == TT - 1
                            nc.tensor.matmul(
                                oa[:, :],
                                v_ext[:, kb, pb : pb + 128],
                                pt[:, 0:QC],
                                start=first,
                                stop=last,
                            )
                            nc.tensor.matmul(
                                ob[:, :],
                                v_ext[:, kb, pb + 32 : pb + 160],
                                pt[:, QC : 2 * QC],
                                start=first,
                                stop=last,
                            )
                            # splice filler into the per-key-block exp hole:
                            # exp (1114ns) exceeds this slot's PE work
                            # (paired scores ~240ns + 2 PV matmuls ~450ns),
                            # leaving ~420ns for one item (~2 matmuls) of
                            # next-pair QK proj (critical) or bonus work.
                            pop_filler(qk_work)
                        # epilogue: copy psum out early (frees oa/ob banks),
                        # then normalize by the ones-row sums
                        osb_a = pool.tile([P, QC], F32, tag="osb_a", bufs=xbufs)
                        osb_b = pool.tile([P, QC], F32, tag="osb_b", bufs=xbufs)
                        nc.vector.tensor_copy(osb_a[0:65, :], oa[0:65, :])
                        nc.vector.tensor_copy(osb_b[64:128, :], ob[64:128, :])
                        nc.vector.tensor_copy(osb_b[32:33, :], ob[32:33, :])
                        # denominators -> DRAM, reshaped to [128, 8] so the
                        # slow iterative DVE reciprocal uses all lanes, then
                        # broadcast back from DRAM (DMA partition-broadcast).
                        dden = dpool.tile([2, QC], F32, tag="dden", bufs=2)
                        nc.sync.dma_start(dden[0:1, :], osb_a[64:65, :])
                        nc.sync.dma_start(dden[1:2, :], osb_b[32:33, :])
                        den_sq = pool.tile([P, 8], F32, tag="den_sq", bufs=2)
                        nc.sync.dma_start(
                            den_sq[:],
                            dden[:].rearrange("a c -> (a c)").rearrange(
                                "(p f) -> p f", p=P
                            ),
                        )
                        rinv_sq = pool.tile([P, 8], F32, tag="rinv_sq", bufs=2)
                        nc.vector.reciprocal(rinv_sq[:], den_sq[:])
                        drin = dpool.tile([2, QC], F32, tag="drin", bufs=2)
                        nc.sync.dma_start(
                            drin[:].rearrange("a c -> (a c)").rearrange(
                                "(p f) -> p f", p=P
                            ),
                            rinv_sq[:],
                        )
                        rb = pool.tile([P, QC], F32, tag="rb", bufs=xbufs)
                        nc.sync.dma_start(
                            rb[0:64, :], drin[0:1, :].to_broadcast((64, QC))
                        )
                        nc.sync.dma_start(
                            rb[64:128, :], drin[1:2, :].to_broadcast((64, QC))
                        )
                        if dbg and b == 0 and po == 0 and qc == 0:
                            nc.sync.dma_start(d_osba[:], osb_a[:])
                            nc.sync.dma_start(d_osbb[:], osb_b[:])
                            nc.sync.dma_start(d_rb[:], rb[:])
                        nc.vector.tensor_mul(
                            ot[0:64, po, qsl], osb_a[0:64, :], rb[0:64, :]
                        )
                        nc.vector.tensor_mul(
                            ot[64:128, po, qsl], osb_b[64:128, :], rb[64:128, :]
                        )

                    drain(qk_work)
                    if po + 1 < NPAIR:
                        qt_t, kt_t = next_tiles

                if dbg and b == 0:
                    nc.sync.dma_start(d_ot[:], ot[:].bitcast(F32))

                # ---- Y = OT^T @ Wo + bo: deferred into the NEXT batch's
                # attention as bonus filler; the last batch's runs inline
                pending_y = make_y_work(b, ot)

            drain_bonus()
            for chain in pending_y:
                while chain:
                    chain.popleft()()

    nc.finalize()
    return nc


def _run(inputs: dict, mm_dtype=None, attn_bf16=True, trace: bool = False, dbg: bool = False):
    # bf16 projections beat f32r: the PE moving port is 2 B/lane/cycle, so
    # f32r streams at 2 cycles/col while bf16 streams at 1 (measured
    # ~434us bf16 vs ~500us f32r same-session; rel err 3.8e-3 vs 1.7e-3,
    # both far under the 2e-2 budget).
    if mm_dtype is None:
        mm_dtype = mybir.dt.bfloat16
    key = (str(mm_dtype), attn_bf16, dbg)
    if key not in _cache:
        # warm=True streams dummy matmuls under the load prologue so a
        # cold (down-clocked) device ramps the PE p-state before real work
        _cache[key] = build(mm_dtype, attn_bf16=attn_bf16, dbg=dbg, warm=True)
    nc = _cache[key]
    return _run_nc(nc, inputs, trace)


def _run_nc(nc, inputs: dict, trace: bool = False):

    x = np.ascontiguousarray(inputs["inputs"], dtype=np.float32)
    shared = {
        k: np.ascontiguousarray(inputs[k], dtype=np.float32)
        for k in ("Wq", "bq", "Wk", "bk", "Wv", "bv", "Wo", "bo")
    }
    in_maps = [
        {"inputs": x[c * BL : (c + 1) * BL], **shared} for c in range(NCORES)
    ]
    res = run_bass_kernel_spmd(nc, in_maps, list(range(NCORES)), trace=trace)
    full = np.concatenate([res.results[c]["out"] for c in range(NCORES)], axis=0)
    return full, res


def kernel(**inputs) -> np.ndarray:
    out, _ = _run(inputs)
    return out



# revision 50
# speedup vs baseline: 1.0673x; 1.0249x over previous
"""Multi-head attention kernel for Trainium2, data-parallel over batch on 8 cores.

Problem: B=16, N=1024, DIM=768, H=12 heads, head_dim=64, fp32.
  q = x@Wq+bq; k = x@Wk+bk; v = x@Wv+bv   (per-head split)
  out = softmax(q k^T / sqrt(DIM)) v      (per head), concat, @Wo + bo

Sharding: batch-parallel. Each core gets 2 batches and all weights; no
collectives. Output gathered by concat.

Per-core layout strategy (per batch of 1024 tokens):
  - XT = x^T  [768 feat, 1024 tok] via PE transposes (fp32 DMA transpose
    unsupported).
  - QT/KT = (x@W + b)^T [768, 1024]: matmul(lhsT=W, rhs=XT). Head h lives on
    partition rows (h%2)*64..: pair p = m-tile p.
  - V natural [1024 tok, 768] via matmul(lhsT=XT, rhs=Wv), stored per-pair
    padded: [Vh0(64) | ones(1) | pad(31) | Vh1(64)] = 160 cols. The shared
    ones column makes PV emit softmax denominators at 32-aligned psum rows:
      h0: lhsT cols [0:128]  -> psum rows 0-63 = O_h0^T, row 64 = denom_h0
      h1: lhsT cols [32:160] -> psum row 32 = denom_h1, rows 64-127 = O_h1^T
         (remaining rows garbage, never read)
  - S^T[key, q] = matmul(lhsT=KT head rows, rhs=QT head rows), contraction 64,
    two heads row-packed in the PE array (partitions 0-63 / 64-127).
  - P^T = exp(SCALE * S^T) on ACT (no max subtraction needed: |SCALE*S| < ~2),
    [128, 1024] ops (2 key-blocks per op) to amortize ACT overhead.
  - O^T normalized by broadcast reciprocal rows, written to OT [768, 1024].
  - Y = matmul(lhsT=OT, rhs=Wo) + bo -> natural [tok, 768], DMA out.

All matmuls (projections and attention) run in bf16 with fp32 psum
accumulation: the PE moving-operand port is 2 B/lane/cycle on TRN2, so bf16
streams 1 col/cycle while f32r needs 2 — bf16 projections measure ~65us
faster end-to-end than f32r (434us vs 500us same-session), at rel err
~3.8e-3 of the output absmax (budget 2e-2). x is cast to bf16 in DRAM and
transposed via the XBAR DMA-transpose, so the PE does no transpose work.
"""

import sys
import types
from collections import deque

sys.path.insert(0, "/opt/trn_rl_repo")

import numpy as np

# Register the axon NTFF profile hook if the image's antenv lacks it (needed
# only when run with trace=True; harmless otherwise).
import antenv  # noqa: F401

if "antenv.axon_hooks" not in sys.modules:
    _hooks_mod = types.ModuleType("antenv.axon_hooks")
    _hooks_mod._hook = None

    def _set_hook(h):
        _hooks_mod._hook = h

    def _get_hook():
        return _hooks_mod._hook

    _hooks_mod.set_axon_ntff_profile_hook = _set_hook
    _hooks_mod.get_axon_ntff_profile_hook = _get_hook
    sys.modules["antenv.axon_hooks"] = _hooks_mod
    try:
        from trn_agent_boot.trn_boot import _ntff_profile_via_ctypes

        _set_hook(_ntff_profile_via_ctypes("/opt/axon/libaxon_pjrt.so"))
    except Exception:
        pass

import concourse.bass_utils as bass_utils

bass_utils.upload_artifacts = lambda tmpdir: f"local:{tmpdir}"  # no bucket creds

import concourse.bacc as bacc
import concourse.mybir as mybir
import concourse.tile as tile
from concourse.bass_utils import run_bass_kernel_spmd
from concourse.masks import make_identity

P = 128
DIM = 768
N_HEADS = 12
HD = 64
N = 1024
B = 16
NCORES = 8
BL = B // NCORES  # batches per core = 2
SCALE = 1.0 / float(np.sqrt(DIM))

KT = DIM // P      # 6 k-tiles of the 768 contraction
TT = N // P        # 8 token tiles per batch
NPAIR = N_HEADS // 2  # 6 head pairs
QC = 512           # query chunk (psum bank, fp32)
PAIRW = 160        # pair block in V_ext: [Vh0(64)|ones(1)|pad(31)|Vh1(64)]

F32 = mybir.dt.float32

_cache = {}


def build(mm_dtype, attn_bf16=True, dbg=False, warm=False):
    nc = bacc.Bacc("TRN2", target_bir_lowering=False, debug=False)

    x = nc.dram_tensor("inputs", [BL, N, DIM], F32, kind="ExternalInput")
    wq = nc.dram_tensor("Wq", [DIM, DIM], F32, kind="ExternalInput")
    bq = nc.dram_tensor("bq", [DIM], F32, kind="ExternalInput")
    wk = nc.dram_tensor("Wk", [DIM, DIM], F32, kind="ExternalInput")
    bk = nc.dram_tensor("bk", [DIM], F32, kind="ExternalInput")
    wv = nc.dram_tensor("Wv", [DIM, DIM], F32, kind="ExternalInput")
    bv = nc.dram_tensor("bv", [DIM], F32, kind="ExternalInput")
    wo = nc.dram_tensor("Wo", [DIM, DIM], F32, kind="ExternalInput")
    bo = nc.dram_tensor("bo", [DIM], F32, kind="ExternalInput")
    out = nc.dram_tensor("out", [BL, N, DIM], F32, kind="ExternalOutput")
    if dbg:
        d_xt = nc.dram_tensor("d_xt", [P, KT, N], F32, kind="ExternalOutput")
        d_vext = nc.dram_tensor("d_vext", [P, TT, NPAIR * PAIRW], F32, kind="ExternalOutput")
        d_qt = nc.dram_tensor("d_qt", [P, N], F32, kind="ExternalOutput")
        d_kt = nc.dram_tensor("d_kt", [P, N], F32, kind="ExternalOutput")
        d_pt0 = nc.dram_tensor("d_pt0", [P, 2 * QC], F32, kind="ExternalOutput")
        d_pt1 = nc.dram_tensor("d_pt1", [P, 2 * QC], F32, kind="ExternalOutput")
        d_osba = nc.dram_tensor("d_osba", [P, QC], F32, kind="ExternalOutput")
        d_osbb = nc.dram_tensor("d_osbb", [P, QC], F32, kind="ExternalOutput")
        d_rb = nc.dram_tensor("d_rb", [P, QC], F32, kind="ExternalOutput")
        d_ot = nc.dram_tensor("d_ot", [P, KT, N], F32, kind="ExternalOutput")

    wq_r = wq.rearrange("(ko ki) m -> ki ko m", ki=P)
    wk_r = wk.rearrange("(ko ki) m -> ki ko m", ki=P)
    wv_r = wv.rearrange("(ko ki) m -> ki ko m", ki=P)
    wo_r = wo.rearrange("(ko ki) m -> ki ko m", ki=P)
    bq_r = bq.rearrange("(ko ki) -> ki ko", ki=P)
    bk_r = bk.rearrange("(ko ki) -> ki ko", ki=P)

    # weights DMA: gpsimd can cast f32 -> f32r/bf16 in flight
    wdma = nc.sync.dma_start if mm_dtype == F32 else nc.gpsimd.dma_start
    attn_dt = mybir.dt.bfloat16 if attn_bf16 else mm_dtype
    xbufs = 3 if mm_dtype == mybir.dt.bfloat16 else 2

    with tile.TileContext(nc) as tc:
        with (
            tc.tile_pool(name="const", bufs=1) as cpool,
            tc.tile_pool(name="work", bufs=1) as pool,
            tc.tile_pool(name="dram", bufs=1, space="DRAM") as dpool,
            tc.tile_pool(name="ps", bufs=1, space="PSUM") as ps,
        ):
            ident = cpool.tile([P, P], F32)
            make_identity(nc, ident)

            if warm:
                # dummy matmuls to hold the DVFS p-state up while the x cast
                # + XBAR transpose + weight-load prologue runs (PE otherwise
                # idles at kernel start and the clock halves). The pipelined
                # per-token-block transposes get real work going ~12us in, so
                # 32 warm matmuls suffice to bridge.
                wrm = cpool.tile([P, QC], mybir.dt.bfloat16)
                nc.vector.memset(wrm[:], 0.0)
                for _ in range(16):
                    wps = ps.tile([P, QC], F32, tag="mm", bufs=2, name="wps")
                    nc.tensor.matmul(
                        wps[:], wrm[:, 0:P], wrm[:], start=True, stop=True
                    )

            bf16_x = mm_dtype == mybir.dt.bfloat16

            # resident weights (full); order by first use. With x loaded on
            # the sync queue (below), the gpsimd cast queue carries ONLY the
            # weights, so wv lands ~9us in and wq/wk by ~23us.
            wq_sb = cpool.tile([P, KT, DIM], mm_dtype)
            wk_sb = cpool.tile([P, KT, DIM], mm_dtype)
            wv_sb = cpool.tile([P, KT, DIM], mm_dtype)
            wo_sb = cpool.tile([P, KT, DIM], mm_dtype)
            for k in range(KT):
                wdma(wv_sb[:, k], wv_r[:, k])
            for k in range(KT):
                wdma(wq_sb[:, k], wq_r[:, k])
                wdma(wk_sb[:, k], wk_r[:, k])
            # b1's x: cast to bf16 in DRAM (gpsimd, after the weights b0's
            # prologue needs) then ONE full-batch XBAR transpose — zero PE
            # cost, lands ~38us, in time for the V-b1 bonus chains. (Per-
            # token-block XBAR transposes run ~10x slower per byte — the
            # write stripes shrink to 256B — so only the full-batch form is
            # usable, and b0 can't wait for it.)
            xbf1 = None
            if mm_dtype == mybir.dt.bfloat16:
                xbf1 = dpool.tile([N, DIM], mybir.dt.bfloat16, name="xbf1")
                for to in range(TT):
                    nc.gpsimd.dma_start(
                        xbf1[to * P : (to + 1) * P, :],
                        x[1, to * P : (to + 1) * P, :],
                    )
            for k in range(KT):
                wdma(wo_sb[:, k], wo_r[:, k])

            bq_sb = cpool.tile([P, KT], F32)
            bk_sb = cpool.tile([P, KT], F32)
            bv_b = cpool.tile([P, DIM], F32)
            bo_b = cpool.tile([P, DIM], F32)

            # V_ext: [tok_inner, tok_outer, pair blocks of PAIRW cols]
            # cols p*PAIRW + [0:64] = V head 2p, +64 = ones, +[96:160] = V 2p+1
            # pad cols stay uninitialized: they only produce garbage psum rows
            # that are never read. Ones col via DVE cast-copy (f32r producer).
            # Double-buffered (one per batch): b1's V projection runs as
            # filler DURING b0's attention, which still reads b0's V.
            ones_src = cpool.tile([P, TT * NPAIR], F32)
            nc.vector.memset(ones_src[:], 1.0)

            def _alloc_vext():
                v_ext = pool.tile(
                    [P, TT, NPAIR * PAIRW], attn_dt, tag="v_ext", bufs=2,
                    name="v_ext",
                )
                ones_cols = v_ext[:].rearrange(
                    "p t (np w) -> p t np w", w=PAIRW
                )[:, :, :, 64:65]
                nc.vector.tensor_copy(
                    ones_cols,
                    ones_src[:].rearrange("p (t np) -> p t np", np=NPAIR)[
                        :, :, :, None
                    ],
                )
                return v_ext

            # XT via PE transposes: x token blocks stream in as plain f32 on
            # the otherwise-idle sync queue (no cast round-trip through DRAM,
            # no slow small-block XBAR transposes); the PE transposes each
            # 128x128 chunk (f32 transpose = 2 cycles/row, ~107ns/chunk, and
            # doubles as DVFS warm-up work) and the DVE copies psum -> xt in
            # bf16. Both batches run here: b1's transposes fill the wv-paced
            # bubbles of b0's V projection, and issuing its x loads late
            # would park them behind b0's attention-epilogue DMAs (FIFO).
            xts = []

            def _transpose_x(b):
                # psum -> xt copies ride the Scalar (ACT) engine, idle until
                # the first exp ~35us in. tp rides the "st" psum tag (also
                # idle until attention): on the "mm" tag its WAR rotation
                # against the V-chain accumulators serializes transpose(to+1)
                # behind V-chain(to)'s scatter, stretching the prologue.
                for to in range(TT):
                    tsl = slice(to * P, (to + 1) * P)
                    xstage = pool.tile([P, DIM], F32, tag="xstage", bufs=3)
                    nc.sync.dma_start(xstage[:], x[b, tsl, :])
                    tp = ps.tile([P, 2 * QC], F32, tag="st", bufs=2, name="tp")
                    for fo in range(KT):
                        nc.tensor.transpose(
                            tp[:, fo * P : (fo + 1) * P],
                            xstage[:, fo * P : (fo + 1) * P],
                            ident,
                        )
                    nc.scalar.activation(
                        xts[b][:, 0:4, tsl],
                        tp[:, 0:512].rearrange("p (f t) -> p f t", f=4),
                        mybir.ActivationFunctionType.Copy,
                    )
                    nc.scalar.activation(
                        xts[b][:, 4:6, tsl],
                        tp[:, 512:768].rearrange("p (f t) -> p f t", f=2),
                        mybir.ActivationFunctionType.Copy,
                    )

            for b in range(BL):
                xt_b = pool.tile(
                    [P, KT, N], mm_dtype, tag="xt_ot", bufs=xbufs, name=f"xt{b}"
                )
                xts.append(xt_b)
            _transpose_x(0)
            if xbf1 is not None:
                # b1's transpose: XBAR, no PE cost. Queued on sync after b0's
                # x loads; it waits on the b1 cast (~30us) while the first
                # attention-epilogue DMA isn't needed until ~50us.
                nc.sync.dma_start_transpose(xts[1][:], xbf1[:])
            else:
                _transpose_x(1)

            # ---- deferred-work machinery ------------------------------------
            # All projection work is expressed as closures ("items") grouped
            # into CHAINS (one psum accumulation each, ~6 items of ~2 matmuls).
            # Chains from the `bonus` queue fill attention exp-holes that the
            # pair-critical QK items don't cover; an open chain is finished
            # before anything else is popped so its psum-tag buffers (tag
            # "mm", 2 bufs) never interleave with another accumulation.
            bonus = deque()
            state = {"chain": None}

            def pop_filler(crit):
                ch = state["chain"]
                if ch is not None:
                    ch.popleft()()
                    if not ch:
                        state["chain"] = None
                    return
                if crit:
                    crit.popleft()()
                    return
                if bonus:
                    chain = bonus.popleft()
                    chain.popleft()()
                    state["chain"] = chain if chain else None

            def drain(crit):
                while state["chain"] is not None or crit:
                    pop_filler(crit)

            def drain_bonus():
                while state["chain"] is not None or bonus:
                    pop_filler([])

            def _accum_tiles(alt, name):
                """Pair of psum accumulators ([P,512] + [P,256]) for a proj
                chain, as (tile, col_offset) pairs. alt=False: two tiles from
                the shared "mm" tag — the chain holds BOTH bufs, so two such
                chains serialize on the WAR rotation (the next chain's
                matmuls wait for this chain's scatter). alt=True: two slices
                of ONE "st"-tag tile (1 of 2 bufs) — such chains overlap
                both each other and any mm-tag chain."""
                if not alt:
                    return {
                        ch: (ps.tile([P, QC], F32, tag="mm", bufs=2,
                                     name=f"{name}{ch}"), 0)
                        for ch in (0, 1)
                    }
                st_t = ps.tile([P, 2 * QC], F32, tag="st", bufs=2, name=name)
                return {0: (st_t, 0), 1: (st_t, QC)}

            def make_v_work(xt_b, vext_b, alt_ok=False):
                """V projection for one batch: one chain per token block
                (6 k-step items of 2 matmuls; the last scatters heads into
                the pair-padded v_ext layout and adds the bias)."""
                chains = []
                for to in range(TT):
                    holder = {}
                    alt = alt_ok and to % 2 == 1

                    def step(k, to=to, holder=holder, alt=alt):
                        if k == 0:
                            holder["vps"] = _accum_tiles(alt, "vps")
                        for ch, cw in ((0, 512), (1, 256)):
                            t, off = holder["vps"][ch]
                            nc.tensor.matmul(
                                t[:, off : off + cw],
                                xt_b[:, k, to * P : (to + 1) * P],
                                wv_sb[:, k, ch * 512 : ch * 512 + cw],
                                start=(k == 0),
                                stop=(k == KT - 1),
                            )
                        if k != KT - 1:
                            return
                        for ch, cw in ((0, 512), (1, 256)):
                            t, off = holder["vps"][ch]
                            npr = cw // (2 * HD)  # pairs in this chunk
                            pr0 = ch * 4
                            for par in (0, 1):  # even/odd head of each pair
                                src = t[:, off : off + cw].rearrange(
                                    "p (np two w) -> p np two w", two=2, w=HD
                                )[:, :, par, :]
                                bsrc = bv_b[:, ch * 512 : ch * 512 + cw].rearrange(
                                    "p (np two w) -> p np two w", two=2, w=HD
                                )[:, :, par, :]
                                off = 96 if par else 0
                                dst = vext_b[:, to, :].rearrange(
                                    "p (np w) -> p np w", w=PAIRW
                                )[:, pr0 : pr0 + npr, off : off + HD]
                                nc.vector.scalar_tensor_tensor(
                                    out=dst,
                                    in0=src,
                                    scalar=1.0,
                                    in1=bsrc,
                                    op0=mybir.AluOpType.mult,
                                    op1=mybir.AluOpType.add,
                                )

                    chains.append(
                        deque(lambda k=k, step=step: step(k) for k in range(KT))
                    )
                return chains

            def make_qk_work(xt_b, npo, nqt, nkt, ptags=("mm", "mm")):
                """QK projection for pair npo: two chains (dst q, dst k) of
                6 k-step items; the last item adds the biases. ptags picks
                the psum tags for the two query-chunk accumulators — pair 0
                of b0 runs pre-attention and borrows the idle oa/ob banks so
                it overlaps the V projection instead of serializing behind
                it on the mm tag's WAR rotation."""
                chains = []
                for dst_t, w_t, bias in ((nqt, wq_sb, bq_sb), (nkt, wk_sb, bk_sb)):
                    holder = {}

                    def chunk(k, dst_t=dst_t, w_t=w_t, bias=bias, holder=holder):
                        if k == 0:
                            holder["pp"] = [
                                ps.tile(
                                    [P, QC], F32, tag=ptags[qs],
                                    bufs=2 if ptags[qs] == "mm" else 1,
                                    name=f"pps{qs}",
                                )
                                for qs in range(N // QC)
                            ]
                        # two concurrent 64-row tile pairs (qs0-lo ∥ qs1-hi,
                        # then qs1-lo ∥ qs0-hi): column-neutral, keeps the
                        # PE in 64-row config so attention slots never pay
                        # the row-group switch drain
                        for qs in range(N // QC):
                            nc.tensor.matmul(
                                holder["pp"][qs][:],
                                w_t[:, k, npo * P : (npo + 1) * P],
                                xt_b[:, k, qs * QC : (qs + 1) * QC],
                                start=(k == 0),
                                stop=(k == KT - 1),
                            )
                        if k == KT - 1:
                            for qs in range(N // QC):
                                nc.vector.tensor_scalar_add(
                                    dst_t[:, qs * QC : (qs + 1) * QC],
                                    holder["pp"][qs][:],
                                    bias[:, npo : npo + 1],
                                )

                    chains.append(
                        deque(lambda k=k, chunk=chunk: chunk(k) for k in range(KT))
                    )
                return chains

            def make_y_work(b, ot_b, alt_all=False):
                """Output projection for one batch: one chain per token block.
                Output DMA rides the gpsimd queue (idle post-prologue) so it
                can't delay the attention-epilogue DMAs on the sync queue.
                alt_all=True (last batch, runs post-attention): accumulators
                live in "st"-tag tiles so the chains overlap the mm-tag
                chains of the previous batch's leftovers AND each other."""
                chains = []
                for to in range(TT):
                    holder = {}

                    def step(k, b=b, ot_b=ot_b, to=to, holder=holder):
                        if k == 0:
                            holder["yps"] = _accum_tiles(alt_all, "yps")
                        for ch, cw in ((0, 512), (1, 256)):
                            t, off = holder["yps"][ch]
                            nc.tensor.matmul(
                                t[:, off : off + cw],
                                ot_b[:, k, to * P : (to + 1) * P],
                                wo_sb[:, k, ch * 512 : ch * 512 + cw],
                                start=(k == 0),
                                stop=(k == KT - 1),
                            )
                        if k != KT - 1:
                            return
                        ystage = pool.tile([P, DIM], F32, tag="ystage", bufs=xbufs)
                        for ch, cw in ((0, 512), (1, 256)):
                            t, off = holder["yps"][ch]
                            nc.vector.scalar_tensor_tensor(
                                out=ystage[:, ch * 512 : ch * 512 + cw],
                                in0=t[:, off : off + cw],
                                scalar=1.0,
                                in1=bo_b[:, ch * 512 : ch * 512 + cw],
                                op0=mybir.AluOpType.mult,
                                op1=mybir.AluOpType.add,
                            )
                        nc.gpsimd.dma_start(
                            out[b, to * P : (to + 1) * P, :], ystage[:]
                        )

                    chains.append(
                        deque(lambda k=k, step=step: step(k) for k in range(KT))
                    )
                return chains

            pending_y = None
            b1_qkt = None
            for b in range(BL):
                xt = xts[b]
                if b == 0:
                    nc.scalar.dma_start(bv_b[:], bv[None, :].to_broadcast((P, DIM)))
                    nc.scalar.dma_start(bq_sb[:], bq_r)
                    nc.scalar.dma_start(bk_sb[:], bk_r)
                    nc.scalar.dma_start(bo_b[:], bo[None, :].to_broadcast((P, DIM)))
                    # ---- V natural (b0, inline) ---------------------------
                    v_ext = _alloc_vext()
                    for chain in make_v_work(xt, v_ext, alt_ok=False):
                        while chain:
                            chain.popleft()()
                    # pair 0's QK projection inline, on the idle oa/ob psum
                    # banks so it overlaps the V projection
                    qt_t = pool.tile([P, N], attn_dt, tag="qt", bufs=4)
                    kt_t = pool.tile([P, N], attn_dt, tag="kt", bufs=4)
                    for chain in make_qk_work(xt, 0, qt_t, kt_t, ("oa", "ob")):
                        while chain:
                            chain.popleft()()
                    # bonus for b0's attention: b1's V projection + b1's
                    # pair-0 QK projection
                    v_ext1 = _alloc_vext()
                    bonus.extend(make_v_work(xts[1], v_ext1))
                    b1_qkt = (
                        pool.tile([P, N], attn_dt, tag="qt", bufs=4, name="qt1"),
                        pool.tile([P, N], attn_dt, tag="kt", bufs=4, name="kt1"),
                    )
                    bonus.extend(make_qk_work(xts[1], 0, *b1_qkt))
                    if dbg:
                        nc.sync.dma_start(d_xt[:], xt[:].bitcast(F32))
                        nc.sync.dma_start(d_vext[:], v_ext[:].bitcast(F32))
                else:
                    # leftover b1 V / QK0 bonus must land before b1's
                    # attention reads v_ext1 / its qt,kt
                    drain_bonus()
                    v_ext = v_ext1
                    qt_t, kt_t = b1_qkt
                    # bonus for b1's attention: b0's output projection
                    bonus.extend(pending_y)

                # ---- OT buffer for this batch -----------------------------
                ot = pool.tile([P, KT, N], mm_dtype, tag="xt_ot", bufs=xbufs, name="ot")

                for po in range(NPAIR):
                    qk_work = deque()
                    if po + 1 < NPAIR:
                        nqt = pool.tile([P, N], attn_dt, tag="qt", bufs=4)
                        nkt = pool.tile([P, N], attn_dt, tag="kt", bufs=4)
                        for chain in make_qk_work(xt, po + 1, nqt, nkt):
                            qk_work.extend(chain)
                        next_tiles = (nqt, nkt)

                    if dbg and b == 0 and po == 0:
                        nc.sync.dma_start(d_qt[:], qt_t[:].bitcast(F32))
                        nc.sync.dma_start(d_kt[:], kt_t[:].bitcast(F32))

                    pb = po * PAIRW
                    for qc in range(N // QC):
                        qsl = slice(qc * QC, (qc + 1) * QC)
                        oa = ps.tile([P, QC], F32, tag="oa", bufs=1, name="oa")
                        ob = ps.tile([P, QC], F32, tag="ob", bufs=1, name="ob")
                        for kb in range(TT):
                            ksl = slice(kb * P, (kb + 1) * P)
                            # ONE st tile per key block holding BOTH heads
                            # ([h0 512 | h1 512]) so a single exp frees both
                            # banks at once: the two score matmuls then become
                            # ready together and issue back-to-back, streaming
                            # CONCURRENTLY on the two 64-row PE tile halves
                            # (h0 rows 0-63, h1 rows 64-127) — 2 cols/cycle.
                            # With separate st0/st1 + two exps, h1's bank
                            # frees ~1.1us after h0's and the scheduler runs
                            # the h0 matmuls solo, breaking the pairing.
                            st = ps.tile([P, 2 * QC], F32, tag="st", bufs=2, name="st")
                            # high_priority keeps the pair adjacent in the
                            # final schedule: filler k-steps become ready
                            # one-at-a-time (chained psum accumulation) and
                            # would otherwise slip between h0 and h64 with
                            # their lower (earlier) priorities.
                            with tc.high_priority(offset=256):
                                nc.tensor.matmul(
                                    st[:, 0:QC],
                                    kt_t[0:64, ksl],
                                    qt_t[0:64, qsl],
                                    start=True,
                                    stop=True,
                                )
                                nc.tensor.matmul(
                                    st[:, QC : 2 * QC],
                                    kt_t[64:128, ksl],
                                    qt_t[64:128, qsl],
                                    start=True,
                                    stop=True,
                                )
                            pt = pool.tile([P, 2 * QC], attn_dt, tag="pt0", bufs=xbufs)
                            nc.scalar.activation(
                                pt[:], st[:], mybir.ActivationFunctionType.Exp,
                                scale=SCALE,
                            )
                            if dbg and b == 0 and po == 0 and qc == 0 and kb == 0:
                                nc.sync.dma_start(d_pt0[:], pt[:].bitcast(F32))
                            first = kb == 0
                            last = kb == TT - 1
                            nc.tensor.matmul(
                                oa[:, :],
                                v_ext[:, kb, pb : pb + 128],
                                pt[:, 0:QC],
                                start=first,
                                stop=last,
                            )
                            nc.tensor.matmul(
                                ob[:, :],
                                v_ext[:, kb, pb + 32 : pb + 160],
                                pt[:, QC : 2 * QC],
                                start=first,
                                stop=last,
                            )
                            # splice filler into the per-key-block exp hole:
                            # exp (1114ns) exceeds this slot's PE work
                            # (paired scores ~240ns + 2 PV matmuls ~450ns),
                            # leaving ~420ns for one item (~2 matmuls) of
                            # next-pair QK proj (critical) or bonus work.
                            pop_filler(qk_work)
                        # epilogue: copy psum out early (frees oa/ob banks),
                        # then normalize by the ones-row sums
                        osb_a = pool.tile([P, QC], F32, tag="osb_a", bufs=xbufs)
                        osb_b = pool.tile([P, QC], F32, tag="osb_b", bufs=xbufs)
                        nc.vector.tensor_copy(osb_a[0:65, :], oa[0:65, :])
                        nc.vector.tensor_copy(osb_b[64:128, :], ob[64:128, :])
                        nc.vector.tensor_copy(osb_b[32:33, :], ob[32:33, :])
                        # denominators -> DRAM, reshaped to [128, 8] so the
                        # slow iterative DVE reciprocal uses all lanes, then
                        # broadcast back from DRAM (DMA partition-broadcast).
                        dden = dpool.tile([2, QC], F32, tag="dden", bufs=2)
                        nc.sync.dma_start(dden[0:1, :], osb_a[64:65, :])
                        nc.sync.dma_start(dden[1:2, :], osb_b[32:33, :])
                        den_sq = pool.tile([P, 8], F32, tag="den_sq", bufs=2)
                        nc.sync.dma_start(
                            den_sq[:],
                            dden[:].rearrange("a c -> (a c)").rearrange(
                                "(p f) -> p f", p=P
                            ),
                        )
                        rinv_sq = pool.tile([P, 8], F32, tag="rinv_sq", bufs=2)
                        nc.vector.reciprocal(rinv_sq[:], den_sq[:])
                        drin = dpool.tile([2, QC], F32, tag="drin", bufs=2)
                        nc.sync.dma_start(
                            drin[:].rearrange("a c -> (a c)").rearrange(
                                "(p f) -> p f", p=P
                            ),
                            rinv_sq[:],
                        )
                        rb = pool.tile([P, QC], F32, tag="rb", bufs=xbufs)
                        nc.sync.dma_start(
                            rb[0:64, :], drin[0:1, :].to_broadcast((64, QC))
                        )
                        nc.sync.dma_start(
                            rb[64:128, :], drin[1:2, :].to_broadcast((64, QC))
                        )
                        if dbg and b == 0 and po == 0 and qc == 0:
                            nc.sync.dma_start(d_osba[:], osb_a[:])
                            nc.sync.dma_start(d_osbb[:], osb_b[:])
                            nc.sync.dma_start(d_rb[:], rb[:])
                        nc.vector.tensor_mul(
                            ot[0:64, po, qsl], osb_a[0:64, :], rb[0:64, :]
                        )
                        nc.vector.tensor_mul(
                            ot[64:128, po, qsl], osb_b[64:128, :], rb[64:128, :]
                        )

                    drain(qk_work)
                    if po + 1 < NPAIR:
                        qt_t, kt_t = next_tiles

                if dbg and b == 0:
                    nc.sync.dma_start(d_ot[:], ot[:].bitcast(F32))

                # ---- Y = OT^T @ Wo + bo: deferred into the NEXT batch's
                # attention as bonus filler; the last batch's runs inline
                pending_y = make_y_work(b, ot, alt_all=(b == BL - 1))

            # tail: round-robin leftover bonus chains (Y-b0, mm tag) with
            # Y-b1 chains (st tag) two at a time — disjoint psum tags let an
            # mm chain and an st chain stream concurrently instead of each
            # chain stalling the next on its scatter
            if state["chain"] is not None:
                while state["chain"] is not None:
                    pop_filler([])
            # Two lanes, each sequential WITHIN itself (interleaving two
            # same-tag chains is a correctness hazard: the second chain's
            # psum alloc only orders against the first's already-emitted
            # readers, racing its remaining accumulation). Y-b0 leftovers
            # (mm tag) and Y-b1 chains (st tag) interleave across lanes.
            lane_mm = deque(bonus)
            bonus.clear()
            lane_st = deque(pending_y)
            cur = [None, None]
            lanes = [lane_mm, lane_st]
            while any(cur) or any(lanes):
                for i in (0, 1):
                    if not cur[i] and lanes[i]:
                        cur[i] = lanes[i].popleft()
                    if cur[i]:
                        cur[i].popleft()()
                        if not cur[i]:
                            cur[i] = None

    nc.finalize()
    return nc


def _run(inputs: dict, mm_dtype=None, attn_bf16=True, trace: bool = False, dbg: bool = False):
    # bf16 projections beat f32r: the PE moving port is 2 B/lane/cycle, so
    # f32r streams at 2 cycles/col while bf16 streams at 1 (measured
    # ~434us bf16 vs ~500us f32r same-session; rel err 3.8e-3 vs 1.7e-3,
    # both far under the 2e-2 budget).
    if mm_dtype is None:
        mm_dtype = mybir.dt.bfloat16
    key = (str(mm_dtype), attn_bf16, dbg)
    if key not in _cache:
        # warm=True streams dummy matmuls under the load prologue so a
        # cold (down-clocked) device ramps the PE p-state before real work
        _cache[key] = build(mm_dtype, attn_bf16=attn_bf16, dbg=dbg, warm=True)
    nc = _cache[key]
    return _run_nc(nc, inputs, trace)


def _run_nc(nc, inputs: dict, trace: bool = False):

    x = np.ascontiguousarray(inputs["inputs"], dtype=np.float32)
    shared = {
        k: np.ascontiguousarray(inputs[k], dtype=np.float32)
        for k in ("Wq", "bq", "Wk", "bk", "Wv", "bv", "Wo", "bo")
    }
    in_maps = [
        {"inputs": x[c * BL : (c + 1) * BL], **shared} for c in range(NCORES)
    ]
    res = run_bass_kernel_spmd(nc, in_maps, list(range(NCORES)), trace=trace)
    full = np.concatenate([res.results[c]["out"] for c in range(NCORES)], axis=0)
    return full, res


def kernel(**inputs) -> np.ndarray:
    out, _ = _run(inputs)
    return out

